# revision 19
# baseline (speedup 1.0000x reference)
"""Trainium2 Bass kernel for nn_Model_39676907882504.

Math: qk = (q @ k^T)/8 has shape [1,2048,1,1]; after the transposes it is
[2048,1,1,1], and softmax over the trailing size-1 axis is exactly 1.0
regardless of qk.  The final matmul with attn_weight == 1 reduces to
broadcasting `value` across a new leading dim:

    output[i, j, 0, :] = value[0, j, 0, :]   for all i in [0, 2048)

i.e. a 512KB -> 1GiB broadcast copy.  Pure memory-regime kernel.
Sharding: 256 output rows per core x 8 cores; value staged in SBUF.

HW model (established by trace analysis + probe kernels this session):
  - For a 2-dim DRAM-side AP, descriptors are assigned to the 16 SDMA
    engines singly round-robin: desc i -> engine 64+(i%16), restarting
    per instruction.  (3-dim DRAM APs switch to packet-per-outer-index
    assignment whose consecutive-partition runs break port affinity -
    avoid.)
  - SBUF AXI port p serves partitions ≡ p (mod 16), so with the SBUF
    partition dim in descriptor order, engine k only ever touches port
    k: zero port contention (measured 26.9 GB/s/engine = 99% of the
    32B x 850MHz port rate).
  - Pipelining needs same-queue descriptor runs: engines alternate
    between the two HWDGE queues at run boundaries, and each switch
    costs ~2.5-4us unless the run is >=8 descs.  8 descs/engine per
    instruction (128-desc instructions) measured bubble-free.
  - Instructions with 1 desc/engine serialize at ~4.5-5.4us/desc: the
    old per-copy loads burned ~35us; a 64-desc load (4 descs/engine)
    pipelines.
  - SBUF AP partition dim caps descriptors at 128/instruction, and
    desc-count ≢ 0 (mod 16) schemes die on the 128-partition wrap, so
    engine 79's ~21% speed deficit (known quirk) cannot be rebalanced
    in this structure; it sets the tail.

Kernel: SBUF tile [128, 8192]: partition q holds row-chunk (q mod 8) =
vflat[8192*(q%16) : +8192] (host-replicated 8x, uploaded before
execution).  Stores: 16 instructions per queue, each [128, 8192] ->
8 output rows (4 MiB, 8 descs/engine).  Loads: one 64-desc instruction
per queue (partitions 0-63 / 64-127).  Final store per queue doubles as
the drain barrier (per-engine FIFO).
"""

import sys

for _p in ("/opt/trn_rl_repo",):
    if _p not in sys.path:
        sys.path.insert(0, _p)

import numpy as np

import concourse.bass as bass
import concourse.mybir as mybir
from concourse.bass_utils import run_bass_kernel_spmd

S = 2048
D = 64
N_CORES = 8
ROWS_PER_CORE = S // N_CORES          # 256
ROW_FL = S * D                        # 131072 floats per output row
CHUNK = 16384                         # floats per descriptor (64 KiB)
RG = 16                               # rows per store instruction

TRACE = False          # test.py flips this to profile
TRACE_KWARGS = {}
LAST_RESULT = None     # BassKernelResults of the last run (for test.py)


def build_program():
    nc = bass.Bass()
    val = nc.declare_dram_parameter("value_r", [128, CHUNK],
                                    mybir.dt.float32, isOutput=False)
    out = nc.declare_dram_parameter("out", [ROWS_PER_CORE, ROW_FL],
                                    mybir.dt.float32, isOutput=True)
    wt = nc.alloc_sbuf_tensor("wt", [128, CHUNK], mybir.dt.float32)

    def store(eng, r0, nrows=RG, p0=0):
        return eng.dma_start(
            out=out[r0:r0 + nrows, 0:ROW_FL].rearrange(
                "r (p c) -> (r p) c", p=8),
            in_=wt[p0:p0 + 8 * nrows, 0:CHUNK])

    half = ROWS_PER_CORE // 2
    n_ins = half // RG                                # 16 per queue

    with nc.Block() as block, nc.semaphore("dma_sem") as dma_sem, \
            nc.semaphore("scr_sem") as scr_sem:

        @block.sync
        def _(sync):
            # L1: tile half A (partitions 0-63); L2: half B - sequential
            # on this queue so the reads stream 8-deep per engine.
            sync.dma_start(out=wt[0:64, :],
                           in_=val[0:64, :]).then_inc(dma_sem, 16)
            sync.dma_start(out=wt[64:128, :],
                           in_=val[64:128, :]).then_inc(dma_sem, 16)
            sync.wait_ge(dma_sem, 32)
            for k in range(1, n_ins):
                ins = store(sync, k * RG)
                ins.then_inc(dma_sem if k == n_ins - 1 else scr_sem, 16)
            # rows 0-15 last (their half-tile twin went out on scalar)
            store(sync, 0, RG // 2, 0).then_inc(scr_sem, 16)
            store(sync, RG // 2, RG // 2, 64).then_inc(dma_sem, 16)
            sync.wait_ge(dma_sem, 80)

        @block.scalar
        def _(scalar):
            # overlap with L2: stores that only touch tile half A
            scalar.wait_ge(dma_sem, 16)
            store(scalar, half, RG // 2, 0).then_inc(scr_sem, 16)
            store(scalar, half + RG // 2, RG // 2, 0).then_inc(scr_sem, 16)
            scalar.wait_ge(dma_sem, 32)
            for k in range(1, n_ins):
                ins = store(scalar, half + k * RG)
                ins.then_inc(dma_sem if k == n_ins - 1 else scr_sem, 16)
            scalar.wait_ge(dma_sem, 80)

    return nc


def kernel(query=None, key=None, value=None, attn_mask=None, **_ignored):
    global LAST_RESULT
    vflat = np.ascontiguousarray(np.asarray(value, np.float32)).reshape(ROW_FL)
    vr = np.tile(vflat.reshape(8, CHUNK), (16, 1))

    nc = build_program()
    core_ids = list(range(N_CORES))
    in_maps = [{"value_r": vr} for _ in core_ids]
    res = run_bass_kernel_spmd(nc, in_maps, core_ids, trace=TRACE,
                               **TRACE_KWARGS)
    LAST_RESULT = res

    shards = [res.results[i]["out"].reshape(ROWS_PER_CORE, S, 1, D)
              for i in range(N_CORES)]
    return np.concatenate(shards, axis=0)


# revision 20
# speedup vs baseline: 1.0786x; 1.0786x over previous
"""Trainium2 Bass kernel for nn_Model_39676907882504.

Math: qk = (q @ k^T)/8 has shape [1,2048,1,1]; after the transposes it is
[2048,1,1,1], and softmax over the trailing size-1 axis is exactly 1.0
regardless of qk.  The final matmul with attn_weight == 1 reduces to
broadcasting `value` across a new leading dim:

    output[i, j, 0, :] = value[0, j, 0, :]   for all i in [0, 2048)

i.e. a 512KB -> 1GiB broadcast copy.  Pure memory-regime kernel.
Sharding: 256 output rows per core x 8 cores; value staged in SBUF.

HW model (established by trace analysis + probe kernels this session):
  - For a 2-dim DRAM-side AP, descriptors are assigned to the 16 SDMA
    engines singly round-robin: desc i -> engine 64+(i%16), restarting
    per instruction.  (3-dim DRAM APs switch to packet-per-outer-index
    assignment whose consecutive-partition runs break port affinity -
    avoid.)
  - SBUF AXI port p serves partitions ≡ p (mod 16), so with the SBUF
    partition dim in descriptor order, engine k only ever touches port
    k: zero port contention (measured 26.9 GB/s/engine = 99% of the
    32B x 850MHz port rate).
  - Pipelining needs same-queue descriptor runs: engines alternate
    between the two HWDGE queues at run boundaries, and each switch
    costs ~2.5-4us unless the run is >=8 descs.  8 descs/engine per
    instruction (128-desc instructions) measured bubble-free.
  - Instructions with 1 desc/engine serialize at ~4.5-5.4us/desc: the
    old per-copy loads burned ~35us; a 64-desc load (4 descs/engine)
    pipelines.
  - SBUF AP partition dim caps descriptors at 128/instruction, and
    desc-count ≢ 0 (mod 16) schemes die on the 128-partition wrap, so
    engine 79's ~21% speed deficit (known quirk) cannot be rebalanced
    in this structure; it sets the tail.

Kernel: SBUF tile [128, 8192]: partition q holds row-chunk (q mod 8) =
vflat[8192*(q%16) : +8192] (host-replicated 8x, uploaded before
execution).  Stores: 16 instructions per queue, each [128, 8192] ->
8 output rows (4 MiB, 8 descs/engine).  Loads: one 64-desc instruction
per queue (partitions 0-63 / 64-127).  Final store per queue doubles as
the drain barrier (per-engine FIFO).
"""

import sys

for _p in ("/opt/trn_rl_repo",):
    if _p not in sys.path:
        sys.path.insert(0, _p)

import numpy as np

import concourse.bass as bass
import concourse.mybir as mybir
from concourse.bass_utils import run_bass_kernel_spmd

S = 2048
D = 64
N_CORES = 8
ROWS_PER_CORE = S // N_CORES          # 256
ROW_FL = S * D                        # 131072 floats per output row
CHUNK = 16384                         # floats per descriptor (64 KiB)
RG = 16                               # rows per store instruction

TRACE = False          # test.py flips this to profile
TRACE_KWARGS = {}
LAST_RESULT = None     # BassKernelResults of the last run (for test.py)


def build_program():
    nc = bass.Bass()
    val = nc.declare_dram_parameter("value_r", [128, CHUNK],
                                    mybir.dt.float32, isOutput=False)
    out = nc.declare_dram_parameter("out", [ROWS_PER_CORE, ROW_FL],
                                    mybir.dt.float32, isOutput=True)
    wt = nc.alloc_sbuf_tensor("wt", [128, CHUNK], mybir.dt.float32)

    def store(eng, r0):
        return eng.dma_start(
            out=out[r0:r0 + RG, 0:ROW_FL].rearrange(
                "r (p c) -> (r p) c", p=8),
            in_=wt[0:128, 0:CHUNK])

    half = ROWS_PER_CORE // 2
    n_ins = half // RG                                # 16 per queue

    with nc.Block() as block, nc.semaphore("dma_sem") as dma_sem, \
            nc.semaphore("scr_sem") as scr_sem:

        @block.sync
        def _(sync):
            sync.dma_start(out=wt[:, :],
                           in_=val[:, :]).then_inc(dma_sem, 16)
            sync.wait_ge(dma_sem, 16)
            for k in range(n_ins):
                ins = store(sync, k * RG)
                ins.then_inc(dma_sem if k == n_ins - 1 else scr_sem, 16)
            sync.wait_ge(dma_sem, 48)

        @block.scalar
        def _(scalar):
            scalar.wait_ge(dma_sem, 16)
            for k in range(n_ins):
                ins = store(scalar, half + k * RG)
                ins.then_inc(dma_sem if k == n_ins - 1 else scr_sem, 16)
            scalar.wait_ge(dma_sem, 48)

    return nc


def kernel(query=None, key=None, value=None, attn_mask=None, **_ignored):
    global LAST_RESULT
    vflat = np.ascontiguousarray(np.asarray(value, np.float32)).reshape(ROW_FL)
    vr = np.tile(vflat.reshape(8, CHUNK), (16, 1))

    nc = build_program()
    core_ids = list(range(N_CORES))
    in_maps = [{"value_r": vr} for _ in core_ids]
    res = run_bass_kernel_spmd(nc, in_maps, core_ids, trace=TRACE,
                               **TRACE_KWARGS)
    LAST_RESULT = res

    shards = [res.results[i]["out"].reshape(ROWS_PER_CORE, S, 1, D)
              for i in range(N_CORES)]
    return np.concatenate(shards, axis=0)
